# revision 31
# baseline (speedup 1.0000x reference)
"""Sliding-window causal GQA attention (RoPE) on 8 TRN2 NeuronCores.

Problem: B=2 packed seqs x S=2048, HQ=32 q heads, HK=8 kv heads, D=128,
WINDOW=1024, causal. GQA group size 4.

Sharding: core c owns kv head c and its 4 query heads (data-parallel over
heads, zero collectives). Per core: flash-style block-sparse attention in
the transposed [kv, q] orientation, bf16 matmuls:
  - per 128-query block, kv span = up to 9 chunks of 128 (window+causal)
  - mm1: S^T[kv,q] = kT_chunk^T . qT_block   (lhsT=kT chunk as weights)
  - exp on ScalarE (attention scale folded into activation scale)
  - triangular masks (bf16 DVE multiplies) on the 2 partial chunks
  - mm2: out[q,d] = sum_c P_c^T . [v_c | 1]  (P chunk as weights; the
    appended ones column accumulates the softmax denominator for free;
    masked chunks are accumulated last so the unmasked middle chunks only
    depend on exp, not on the DVE mask multiplies)
  - normalize by reciprocal(denominator), DMA out in natural [q,d] layout
    via one batched store per (batch, head)
RoPE applied on-device (DVE) in transposed layout with host-precomputed
cos/sin caches built from `positions` (standard rotary tables); loads and
ropes are split per packed sequence and their emission is interleaved two
head-loops ahead of their consumers, so the main loop starts as soon as
k/q0 of sequence 0 are ready and later ropes fill DVE idle slots.
Measured on 8 axon trn2 cores: ~197-200 us NEFF exec, rel_err 4.5e-3 vs
the fp32 reference.
"""

import json
import os
import sys

import numpy as np

sys.path.insert(0, "/opt/trn_rl_repo")

import ml_dtypes  # noqa: E402

import concourse.bass as bass  # noqa: E402
import concourse.tile as tile  # noqa: E402
from concourse import mybir  # noqa: E402
from concourse.bass_utils import run_bass_kernel_spmd  # noqa: E402


# ---------------------------------------------------------------------------
# BIR legalization: this environment's walrus build encodes at most ONE sync
# wait (and one update) per instruction.  Tile attaches several.  Hoist the
# extras onto standalone EventSemaphore nops (same engine, just before the
# owning instruction) — identical semantics, raw-bass style.
# ---------------------------------------------------------------------------
def _legalize_bir(bir_json):
    d = json.loads(bir_json)
    for fn in d["functions"]:
        for blk in fn["blocks"]:
            new = []
            for inst in blk["instructions"]:
                si = inst.get("sync_info")
                if si:
                    waits = si.get("on_wait") or []
                    if len(waits) > 1:
                        for j, w in enumerate(waits[:-1]):
                            new.append({
                                "debug": inst.get("debug", 0),
                                "engine": inst["engine"],
                                "ins": [],
                                "outs": [],
                                "name": f"{inst['name']}_hw{j}",
                                "opcode": "EventSemaphore",
                                "sync_info": {"on_update": [], "on_wait": [w]},
                            })
                        si["on_wait"] = [waits[-1]]
                new.append(inst)
            blk["instructions"] = new
    return json.dumps(d).encode()


def _install_legalizer():
    import concourse.bass_utils as _bu
    import concourse.bass2jax as _b2j

    if getattr(_bu, "_single_wait_legalizer", None):
        return
    _orig = _bu.compile_bir_kernel

    def _patched(bir_json, tmpdir, neff_name="file.neff"):
        return _orig(_legalize_bir(bir_json), tmpdir, neff_name=neff_name)

    _bu.compile_bir_kernel = _patched
    _b2j.compile_bir_kernel = _patched
    _bu._single_wait_legalizer = True


_install_legalizer()

BF16 = ml_dtypes.bfloat16

# Problem config (hardcoded per spec)
B, S = 2, 2048
HQ, HK, D = 32, 8, 128
G = HQ // HK  # 4
WINDOW = 1024
THETA = 10000.0
NTOK = B * S  # 4096
NCORES = 8
HALF = D // 2  # 64

NQB = S // 128          # 16 query blocks of 128 per sequence
NKC = S // 128          # 16 kv chunks of 128 per sequence
MAXCH = WINDOW // 128 + 1  # 9: max kv chunks touched by one q block
SCALE = 1.0 / float(np.sqrt(D))

_CACHED_NC = None


def _build_nc():
    """Build the per-core Bass graph (identical on all 8 cores)."""
    fp32 = mybir.dt.float32
    bf16 = mybir.dt.bfloat16
    nc = bass.Bass()

    q_ext = nc.declare_dram_parameter("q", [NTOK, G * D], bf16, isOutput=False)
    k_ext = nc.declare_dram_parameter("k", [NTOK, D], bf16, isOutput=False)
    v_ext = nc.declare_dram_parameter("v", [NTOK, D + 1], bf16, isOutput=False)
    cos_ext = nc.declare_dram_parameter("cosd", [D, NTOK], bf16, isOutput=False)
    sin_ext = nc.declare_dram_parameter("sind", [D, NTOK], bf16, isOutput=False)
    ctri_ext = nc.declare_dram_parameter("ctri", [128, 128], bf16, isOutput=False)
    wtri_ext = nc.declare_dram_parameter("wtri", [128, 128], bf16, isOutput=False)
    out_ext = nc.declare_dram_parameter("out", [NTOK, G * D], fp32, isOutput=True)

    with tile.TileContext(nc) as tc:
        from contextlib import ExitStack

        with ExitStack() as ctx:
            const = ctx.enter_context(tc.tile_pool(name="const", bufs=1))
            ropet = ctx.enter_context(tc.tile_pool(name="ropet", bufs=2))
            pt_pool = ctx.enter_context(tc.tile_pool(name="pt", bufs=4))
            ep_pool = ctx.enter_context(tc.tile_pool(name="ep", bufs=2))
            st_pool = ctx.enter_context(tc.tile_pool(name="st", bufs=2, space="PSUM"))
            po_pool = ctx.enter_context(tc.tile_pool(name="po", bufs=2, space="PSUM"))

            # ---- persistent SBUF tensors ----
            # per-(g, b) tiles so dependencies stay per-sequence-half
            qTs = [[const.tile([128, S], bf16, name=f"qT{g}b{b}", tag=f"qT{g}b{b}")
                    for b in range(B)] for g in range(G)]
            kTs = [const.tile([128, S], bf16, name=f"kTb{b}", tag=f"kTb{b}")
                   for b in range(B)]
            vsb = const.tile([128, NTOK // 128, D + 1], bf16)  # [kv_in_chunk, chunk, d|1]
            cos_sb = const.tile([128, NTOK], bf16)
            sin_sb = const.tile([128, NTOK], bf16)
            ctri = const.tile([128, 128], bf16)
            wtri = const.tile([128, 128], bf16)

            # ---- loads ----
            # Sync (HWDGE) runs ONLY xbar transposes (mixing DMACopy with
            # DMATranspose on one engine triggers Tile's xbar-mode
            # serialization); every plain copy goes through GpSimd SWDGE.
            nc.gpsimd.dma_start(cos_sb, cos_ext[:, :])
            nc.gpsimd.dma_start(sin_sb, sin_ext[:, :])

            # ---- RoPE in transposed layout ----
            # roped = x * cos_dup + rot(x) * sin_signed
            # rot(x): rows 0:64 <- x[64:128], rows 64:128 <- x[0:64]
            # The rot sbuf->sbuf copies stay on the SAME Sync engine as the
            # xbar transposes: concurrent DMA-transpose + SBUF->SBUF DMA is
            # a documented HW deadlock, and two parallel xbar transposes
            # race; Tile's same-engine mode serialization is the safety.
            def load_rope(x, src, b):
                nc.sync.dma_start_transpose(x, src)
                xr = ropet.tile([128, S], bf16, tag="xr")
                nc.sync.dma_start(xr[0:HALF, :], x[HALF:D, :])
                nc.sync.dma_start(xr[HALF:D, :], x[0:HALF, :])
                t = ropet.tile([128, S], bf16, tag="t")
                cs = cos_sb[:, b * S:(b + 1) * S]
                sn = sin_sb[:, b * S:(b + 1) * S]
                nc.vector.tensor_mul(t, x, cs)
                nc.vector.tensor_mul(xr, xr, sn)
                nc.vector.tensor_add(x, t, xr)

            load_rope(kTs[0][:, :], k_ext[0:S, :], 0)
            load_rope(qTs[0][0][:, :], q_ext[0:S, 0:D], 0)
            nc.gpsimd.dma_start(vsb, v_ext.rearrange("(c p) d -> p c d", p=128))
            nc.gpsimd.dma_start(ctri, ctri_ext[:, :])
            nc.gpsimd.dma_start(wtri, wtri_ext[:, :])

            def rope_for(bj, gj):
                # emit the load+rope for head gj of sequence bj (2 bg-loops
                # ahead of its consumer, so it fills that window's DVE idle)
                if gj == 0:
                    load_rope(kTs[bj][:, :], k_ext[bj * S:(bj + 1) * S, :], bj)
                load_rope(qTs[gj][bj][:, :],
                          q_ext[bj * S:(bj + 1) * S, gj * D:(gj + 1) * D], bj)

            # ---- main attention loop ----
            variant = os.environ.get("KVAR", "full")
            if variant == "rope":
                bg_list = []
            elif variant == "bg1":
                bg_list = [(0, 0)]
            else:
                bg_list = [(b, g) for b in range(B) for g in range(G)]
            for bg_i, (b, g) in enumerate(bg_list):
                osb = ep_pool.tile([128, NQB, D], fp32, tag="osb")
                for qi in range(NQB):
                    if bg_i == 0 and qi == 4 and len(bg_list) > 1:
                        rope_for(0, 1)
                    c0 = max(0, qi - (MAXCH - 1))
                    nch = qi - c0 + 1

                    st = st_pool.tile([128, MAXCH * 128], fp32, tag="st")
                    for ci in range(nch):
                        c = c0 + ci
                        nc.tensor.matmul(
                            st[:, ci * 128:(ci + 1) * 128],
                            kTs[b][:, c * 128:(c + 1) * 128],
                            qTs[g][b][:, qi * 128:(qi + 1) * 128],
                            start=True,
                            stop=True,
                        )

                    pt = pt_pool.tile([128, MAXCH * 128], bf16, tag="pt")
                    nc.scalar.activation(
                        pt[:, : nch * 128],
                        st[:, : nch * 128],
                        mybir.ActivationFunctionType.Exp,
                        scale=SCALE,
                    )
                    # window-partial chunk (first) and causal diagonal (last)
                    if qi >= MAXCH - 1:
                        nc.vector.tensor_mul(pt[:, 0:128], pt[:, 0:128], wtri)
                    nc.vector.tensor_mul(
                        pt[:, (nch - 1) * 128: nch * 128],
                        pt[:, (nch - 1) * 128: nch * 128],
                        ctri,
                    )

                    po = po_pool.tile([128, D + 1], fp32, tag="po")
                    # masked chunks last: the middle chunks depend only on
                    # exp, so mm2 can start before the DVE tri-masks land
                    order = list(range(1 if qi >= MAXCH - 1 else 0, nch - 1))
                    order += [nch - 1] + ([0] if qi >= MAXCH - 1 else [])
                    for j, ci in enumerate(order):
                        c = c0 + ci
                        nc.tensor.matmul(
                            po,
                            pt[:, ci * 128:(ci + 1) * 128],
                            vsb[:, b * (S // 128) + c, :],
                            start=(j == 0),
                            stop=(j == nch - 1),
                        )

                    rec = ep_pool.tile([128, 1], fp32, tag="rec")
                    nc.vector.reciprocal(rec, po[:, D:D + 1])
                    nc.vector.tensor_scalar_mul(osb[:, qi, :], po[:, 0:D], rec)
                # one batched store per (b, g): [128, 16, 128] -> [2048, 128]
                nc.sync.dma_start(
                    out_ext[b * S:(b + 1) * S, g * D:(g + 1) * D]
                    .rearrange("(qi p) d -> p qi d", p=128),
                    osb,
                )
                nj = bg_i + 2
                if nj < len(bg_list):
                    rope_for(*bg_list[nj])

    return nc


def _get_nc():
    global _CACHED_NC
    if _CACHED_NC is None:
        _CACHED_NC = _build_nc()
    return _CACHED_NC


def _host_tables(positions):
    """Rotary cos/sin caches in transposed-dup layout + triangular masks."""
    pos = positions.astype(np.float32)  # [NTOK]
    invf = (1.0 / (THETA ** (np.arange(HALF, dtype=np.float32) / HALF)))  # [64]
    ang = pos[None, :] * invf[:, None]  # [64, NTOK]
    c = np.cos(ang)
    s = np.sin(ang)
    cosd = np.concatenate([c, c], axis=0).astype(BF16)          # [128, NTOK]
    sind = np.concatenate([-s, s], axis=0).astype(BF16)         # [128, NTOK]
    p = np.arange(128)[:, None]
    f = np.arange(128)[None, :]
    ctri = (p <= f).astype(BF16)   # causal diagonal chunk: keep j<=i
    wtri = (f < p).astype(BF16)    # window edge chunk: keep i-j<WINDOW
    return cosd, sind, ctri, wtri


def _run(inputs, trace=False):
    query = inputs["query"]
    key = inputs["key"]
    value = inputs["value"]
    positions = inputs["positions"]

    cosd, sind, ctri, wtri = _host_tables(positions)
    qb = query.astype(BF16)
    kb = key.astype(BF16)
    vb = value.astype(BF16)
    ones = np.ones((NTOK, 1), dtype=BF16)

    in_maps = []
    for c in range(NCORES):
        in_maps.append({
            "q": np.ascontiguousarray(qb[:, c * G * D:(c + 1) * G * D]),
            "k": np.ascontiguousarray(kb[:, c * D:(c + 1) * D]),
            "v": np.ascontiguousarray(
                np.concatenate([vb[:, c * D:(c + 1) * D], ones], axis=1)
            ),
            "cosd": cosd,
            "sind": sind,
            "ctri": ctri,
            "wtri": wtri,
        })

    nc = _get_nc()
    res = run_bass_kernel_spmd(nc, in_maps, core_ids=list(range(NCORES)),
                               trace=trace)
    out = np.concatenate([res.results[c]["out"] for c in range(NCORES)], axis=1)
    return out.astype(np.float32), res


def kernel(query, key, value, positions):
    out, _ = _run({"query": query, "key": key, "value": value,
                   "positions": positions},
                  trace=bool(os.environ.get("KERNEL_TRACE")))
    return out
